# revision 1
# baseline (speedup 1.0000x reference)
"""Trainium2 Bass kernel for nn_MedPoseAttention (multi-head cross-attention).

Full inputs in, full outputs out. Sharding: 8 cores = 4 batches x 2 query-row
halves. Each core computes one batch's K/V projections over the full context
(replicated within the pair -- cheaper than any collective on this hardware)
and attention + output projection for its 512 query rows, all 16 heads.
No cross-core communication.

Per-core dataflow (all matmuls fp32r / tf32-rate):
  qT/kT  = W.T @ X^T per head-pair          [128=2x64hd, L]   (weights stationary)
  scoresT = kT.T @ qT per head              [kv, q] via 2-head row-tiling (K=64)
  expT   = exp(0.125 * scoresT)             ScalarE, PSUM->SBUF, fp32r
  pv     = [v | 1].T @ expT                 M=65: rows 0-63 = head_out^T,
                                            row 64 = softmax denominator
  multiT = pv[0:64] * bcast(1/denom)        DVE mult, gpsimd partition_broadcast
  out    = multiT.T @ Wo + bo               multiT stationary, bias via K=1 matmul
"""

import sys

if "/opt/trn_rl_repo" not in sys.path:
    sys.path.insert(0, "/opt/trn_rl_repo")

import numpy as np

import concourse.bass as bass  # noqa: F401
import concourse.mybir as mybir
from concourse import bacc, tile
from concourse.bass_utils import run_bass_kernel_spmd

F32 = mybir.dt.float32
F32R = mybir.dt.float32r
MULT = mybir.AluOpType.mult
EXP = mybir.ActivationFunctionType.Exp

B, L, D, H, HD = 4, 1024, 1024, 16, 64
NCORES = 8
LQ_C = 512  # query rows per core
NP = H // 2  # head pairs
SCALE = 0.125  # 1/sqrt(HD)

_PROGRAM = None


def build_program(reps=1):
    nc = bacc.Bacc("TRN2", target_bir_lowering=False, debug=False, num_devices=NCORES)

    xqt = nc.dram_tensor("xqt", [D, LQ_C], F32, kind="ExternalInput").ap()
    xct = nc.dram_tensor("xct", [D, L], F32, kind="ExternalInput").ap()
    wq = nc.dram_tensor("wq", [D, H * HD], F32, kind="ExternalInput").ap()
    wk = nc.dram_tensor("wk", [D, H * HD], F32, kind="ExternalInput").ap()
    wv = nc.dram_tensor("wv", [D, H * 65], F32, kind="ExternalInput").ap()
    wo = nc.dram_tensor("wo", [H * HD, D], F32, kind="ExternalInput").ap()
    bq = nc.dram_tensor("bq", [128, NP], F32, kind="ExternalInput").ap()
    bk = nc.dram_tensor("bk", [128, NP], F32, kind="ExternalInput").ap()
    bvrow = nc.dram_tensor("bvrow", [1, H * 65], F32, kind="ExternalInput").ap()
    borow = nc.dram_tensor("borow", [1, D], F32, kind="ExternalInput").ap()
    onesd = nc.dram_tensor("onesd", [1, 128], F32, kind="ExternalInput").ap()

    out = nc.dram_tensor("out", [LQ_C, D], F32, kind="ExternalOutput").ap()
    resid = nc.dram_tensor("resid", [H * HD, LQ_C], F32, kind="ExternalOutput").ap()

    # DRAM views with the d/hc blocking on the partition axis: [128, 8, m]
    wq_v = wq.rearrange("(db p) m -> p db m", p=128)
    wk_v = wk.rearrange("(db p) m -> p db m", p=128)
    wv_v = wv.rearrange("(db p) m -> p db m", p=128)
    wo_v = wo.rearrange("(hb p) m -> p hb m", p=128)

    with tile.TileContext(nc) as tc:
        with (
            tc.tile_pool(name="persist", bufs=1) as persist,
            tc.tile_pool(name="wqk", bufs=3) as wqk_pool,
            tc.tile_pool(name="wbig", bufs=2) as wbig_pool,
            tc.tile_pool(name="qtp", bufs=2) as qt_pool,
            tc.tile_pool(name="ktp", bufs=2) as kt_pool,
            tc.tile_pool(name="expp", bufs=7) as exp_pool,
            tc.tile_pool(name="smallp", bufs=2) as small,
            tc.tile_pool(name="psA", bufs=2, space="PSUM") as psA,
            tc.tile_pool(name="psS", bufs=2, space="PSUM") as psS,
            tc.tile_pool(name="psV", bufs=2, space="PSUM") as psV,
        ):
            def emit_preamble():
                ones_r = persist.tile([1, 128], F32R, tag="ones", name="ones_r")
                bq_sb = persist.tile([128, NP], F32, tag="bq", name="bq_sb")
                bk_sb = persist.tile([128, NP], F32, tag="bk", name="bk_sb")
                bvr = persist.tile([1, H * 65], F32R, tag="bvr", name="bvr")
                bor = persist.tile([1, D], F32R, tag="bor", name="bor")
                xq_all = persist.tile([128, 8, LQ_C], F32R, tag="xq", name="xq_all")
                xqt_v = xqt.rearrange("(db p) m -> p db m", p=128).bitcast(F32R)
                nc.scalar.dma_start(xq_all[:, 0:4, :], xqt_v[:, 0:4, :])
                xq_t = [xq_all[:, d, :] for d in range(8)]
                multi = [
                    persist.tile([128, LQ_C], F32R, tag=f"mt{p}", name=f"mt{p}")
                    for p in range(NP)
                ]
                xc_all = persist.tile([128, 8, L], F32R, tag="xc", name="xc_all")
                v_all = persist.tile([128, 8 * H * 65], F32R, tag="vb", name="v_all")
                return ones_r, bq_sb, bk_sb, bvr, bor, xq_t, multi, xc_all, v_all, xq_all, xqt_v

            def emit_iteration():
                (
                    ones_r, bq_sb, bk_sb, bvr, bor, xq_t, multi, xc_all, v_all,
                    xq_all, xqt_v,
                ) = emit_preamble()
                qT = [None] * NP
                kT = [None] * NP
                expT = [[None] * 8 for _ in range(NP)]
                xc_t = []
                v_buf = []

                def emit_small_consts():
                    nc.sync.dma_start(bq_sb[:], bq[:])
                    nc.sync.dma_start(bk_sb[:], bk[:])

                def emit_late_consts():
                    nc.sync.dma_start(ones_r[:], onesd[:].bitcast(F32R))
                    nc.sync.dma_start(bvr[:], bvrow[:].bitcast(F32R))
                    nc.sync.dma_start(bor[:], borow[:].bitcast(F32R))

                def emit_q(p):
                    wqt = wqk_pool.tile([128, 8, 128], F32R, tag="wqk", name=f"wq{p}")
                    nc.sync.dma_start(wqt[:], wq_v[:, :, p * 128 : (p + 1) * 128].bitcast(F32R))
                    nc.scalar.dma_start(
                        xq_all[:, 4:8, :], xqt_v[:, 4:8, :]
                    )
                    qps = psA.tile([128, LQ_C], F32, tag="proj", name=f"qps{p}")
                    for d in range(8):
                        nc.tensor.matmul(
                            qps[:],
                            lhsT=wqt[:, d, :],
                            rhs=xq_t[d][:],
                            start=(d == 0),
                            stop=(d == 7),
                        )
                    finish_q(p, qps)

                def finish_q(p, qps):
                    qT[p] = qt_pool.tile([128, LQ_C], F32R, tag="qt", name=f"qt{p}")
                    nc.vector.tensor_scalar_add(qT[p][:], qps[:], bq_sb[:, p : p + 1])
                    nc.sync.dma_start(
                        resid[p * 128 : (p + 1) * 128, :], qT[p][:].bitcast(F32)
                    )

                def emit_xct():
                    # context^T split across the gpsimd/SWDGE and scalar/HWDGE
                    # rings, in two merged DMAs.
                    xct_v = xct.rearrange("(db p) m -> p db m", p=128).bitcast(F32R)
                    nc.gpsimd.dma_start(xc_all[:, 0:2, :], xct_v[:, 0:2, :])
                    nc.gpsimd.dma_start(xc_all[:, 2:4, :], xct_v[:, 2:4, :])
                    nc.scalar.dma_start(xc_all[:, 4:6, :], xct_v[:, 4:6, :])
                    nc.scalar.dma_start(xc_all[:, 6:8, :], xct_v[:, 6:8, :])
                    xc_t.extend(xc_all[:, d, :] for d in range(8))
                    v_buf.extend(
                        v_all[:, kvb * H * 65 : (kvb + 1) * H * 65] for kvb in range(8)
                    )

                def emit_k(p):
                    wkt = wqk_pool.tile([128, 8, 128], F32R, tag="wqk", name=f"wk{p}")
                    nc.sync.dma_start(wkt[:], wk_v[:, :, p * 128 : (p + 1) * 128].bitcast(F32R))
                    kT[p] = kt_pool.tile([128, L], F32R, tag="kt", name=f"kt{p}")
                    for ch in range(2):
                        kps = psA.tile([128, 512], F32, tag="proj", name=f"kps{p}{ch}")
                        for d in range(8):
                            nc.tensor.matmul(
                                kps[:],
                                lhsT=wkt[:, d, :],
                                rhs=xc_t[d][:, ch * 512 : (ch + 1) * 512],
                                start=(d == 0),
                                stop=(d == 7),
                            )
                        nc.vector.tensor_scalar_add(
                            kT[p][:, ch * 512 : (ch + 1) * 512], kps[:], bk_sb[:, p : p + 1]
                        )

                def emit_norm(p, h, pvp):
                    dstage = small.tile([65, 512], F32, tag="dstage", name=f"ds{p}{h}", bufs=1)
                    nc.vector.tensor_copy(dstage[64:65, :], pvp[64:65, :])
                    drow = small.tile([1, 512], F32, tag="drow", name=f"dr{p}{h}", bufs=1)
                    nc.sync.dma_start(drow[:], dstage[64:65, :])
                    rrow = small.tile([1, 512], F32R, tag="rrow", name=f"rr{p}{h}", bufs=1)
                    nc.vector.reciprocal(rrow[:], drow[:])
                    R = small.tile([64, 512], F32R, tag="R", name=f"R{p}{h}")
                    nc.gpsimd.partition_broadcast(R[:], rrow[:])
                    if h == 0:
                        nc.vector.tensor_tensor(
                            multi[p][0:64, :], pvp[0:64, :], R[:], op=MULT
                        )
                    else:
                        tmp = small.tile([64, 512], F32R, tag="tmp", name=f"tp{p}{h}")
                        nc.vector.tensor_tensor(tmp[:], pvp[0:64, :], R[:], op=MULT)
                        nc.sync.dma_start(multi[p][64:128, :], tmp[:])

                def emit_sblock(p, pv_pair=None, next_pair=None, pv2_pair=None):
                    pvt = []
                    pvt2 = []
                    if pv2_pair is not None:
                        # last pair's PV rides in psA (idle: no next-pair
                        # projections) at a 2-block lag behind exp
                        pvt2 = [
                            psA.tile([128, LQ_C], F32, tag="proj", name=f"pv{pv2_pair}{h}")
                            for h in range(2)
                        ]
                    if pv_pair is not None:
                        pvt = [
                            psV.tile([128, LQ_C], F32, tag="pv", name=f"pv{pv_pair}{h}")
                            for h in range(2)
                        ]
                    if next_pair is not None:
                        nx = next_pair
                        wqt = wqk_pool.tile([128, 8, 128], F32R, tag="wqk", name=f"wq{nx}")
                        nc.sync.dma_start(
                            wqt[:], wq_v[:, :, nx * 128 : (nx + 1) * 128].bitcast(F32R)
                        )
                        wkt = wqk_pool.tile([128, 8, 128], F32R, tag="wqk", name=f"wk{nx}")
                        nc.sync.dma_start(
                            wkt[:], wk_v[:, :, nx * 128 : (nx + 1) * 128].bitcast(F32R)
                        )
                        qps = psA.tile([128, LQ_C], F32, tag="proj", name=f"qps{nx}")
                        kT[nx] = kt_pool.tile([128, L], F32R, tag="kt", name=f"kt{nx}")
                        kps = [
                            psA.tile([128, 512], F32, tag="proj", name=f"kps{nx}{c}")
                            for c in range(2)
                        ]
                    for kvb in range(8):
                        sps = psS.tile([128, 1024], F32, tag="sps", name=f"sps{p}{kvb}")
                        nc.tensor.matmul(
                            sps[:, 0:512],
                            lhsT=kT[p][0:64, kvb * 128 : (kvb + 1) * 128],
                            rhs=qT[p][0:64, :],
                            start=True,
                            stop=True,
                            tile_position=(0, 0),
                        )
                        nc.tensor.matmul(
                            sps[:, 512:1024],
                            lhsT=kT[p][64:128, kvb * 128 : (kvb + 1) * 128],
                            rhs=qT[p][64:128, :],
                            start=True,
                            stop=True,
                            tile_position=(64, 0),
                        )
                        et = exp_pool.tile([128, 1024], F32R, tag="expt", name=f"et{p}{kvb}")
                        nc.scalar.activation(et[:], sps[:], EXP, scale=SCALE)
                        expT[p][kvb] = et
                        if pv_pair is not None:
                            for h in range(2):
                                hg = 2 * pv_pair + h
                                nc.tensor.matmul(
                                    pvt[h][0:65, :],
                                    lhsT=v_buf[kvb][:, hg * 65 : hg * 65 + 65],
                                    rhs=expT[pv_pair][kvb][:, h * 512 : (h + 1) * 512],
                                    start=(kvb == 0),
                                    stop=(kvb == 7),
                                )
                        if pv2_pair is not None and kvb >= 2:
                            for h in range(2):
                                hg2 = 2 * pv2_pair + h
                                nc.tensor.matmul(
                                    pvt2[h][0:65, :],
                                    lhsT=v_buf[kvb - 2][:, hg2 * 65 : hg2 * 65 + 65],
                                    rhs=expT[pv2_pair][kvb - 2][:, h * 512 : (h + 1) * 512],
                                    start=(kvb == 2),
                                    stop=False,
                                )
                        if next_pair is not None:
                            # one q matmul + two k matmuls of the next pair per step
                            nc.tensor.matmul(
                                qps[:],
                                lhsT=wqt[:, kvb, :],
                                rhs=xq_t[kvb][:],
                                start=(kvb == 0),
                                stop=(kvb == 7),
                            )
                            for j in range(2):
                                ch, d = divmod(2 * kvb + j, 8)
                                ch, d = (0, 2 * kvb + j) if kvb < 4 else (1, 2 * kvb + j - 8)
                                nc.tensor.matmul(
                                    kps[ch][:],
                                    lhsT=wkt[:, d, :],
                                    rhs=xc_t[d][:, ch * 512 : (ch + 1) * 512],
                                    start=(d == 0),
                                    stop=(d == 7),
                                )
                                if d == 7:
                                    nc.vector.tensor_scalar_add(
                                        kT[nx][:, ch * 512 : (ch + 1) * 512],
                                        kps[ch][:],
                                        bk_sb[:, nx : nx + 1],
                                    )
                    if next_pair is not None:
                        finish_q(nx, qps)
                    if pv_pair is not None:
                        for h in range(2):
                            emit_norm(pv_pair, h, pvt[h])
                    if pv2_pair is not None:
                        for kk in (6, 7):
                            for h in range(2):
                                hg2 = 2 * pv2_pair + h
                                nc.tensor.matmul(
                                    pvt2[h][0:65, :],
                                    lhsT=v_buf[kk][:, hg2 * 65 : hg2 * 65 + 65],
                                    rhs=expT[pv2_pair][kk][:, h * 512 : (h + 1) * 512],
                                    start=False,
                                    stop=(kk == 7),
                                )
                        for h in (1, 0):
                            emit_norm(pv2_pair, h, pvt2[h])

                def emit_vproj():
                    # Wv is host-augmented to [D, 16*65]: per head 64 cols + a zero
                    # column whose bias is 1.0 -> projection emits [v | 1] slots
                    # directly, softmax denominators ride the PV matmul for free.
                    for ch in range(4):
                        wvt = wbig_pool.tile([128, 8, 260], F32R, tag="wbig", name=f"wv{ch}")
                        nc.sync.dma_start(
                            wvt[:], wv_v[:, :, ch * 260 : (ch + 1) * 260].bitcast(F32R)
                        )
                        for kvb in range(8):
                            vps = psA.tile([128, 260], F32, tag="proj", name=f"vps{ch}{kvb}")
                            for d in range(8):
                                nc.tensor.matmul(
                                    vps[:],
                                    lhsT=xc_t[d][:, kvb * 128 : (kvb + 1) * 128],
                                    rhs=wvt[:, d, :],
                                    start=(d == 0),
                                    stop=False,
                                )
                            nc.tensor.matmul(
                                vps[:],
                                lhsT=ones_r[0:1, :],
                                rhs=bvr[0:1, ch * 260 : (ch + 1) * 260],
                                start=False,
                                stop=True,
                            )
                            nc.vector.tensor_copy(
                                v_all[:, kvb * 1040 + ch * 260 : kvb * 1040 + (ch + 1) * 260],
                                vps[:],
                            )

                wo_t = [None, None]

                def emit_wo_loads():
                    for ch in range(2):
                        wo_t[ch] = wbig_pool.tile(
                            [128, 8, 512], F32R, tag="wbig", name=f"wo{ch}"
                        )
                        nc.gpsimd.dma_start(
                            wo_t[ch][:], wo_v[:, :, ch * 512 : (ch + 1) * 512].bitcast(F32R)
                        )

                def emit_pv(p):
                    for h in (1, 0):
                        hg = 2 * p + h
                        pvp = psV.tile([128, LQ_C], F32, tag="pv", name=f"pv{p}{h}")
                        for kvb in range(8):
                            nc.tensor.matmul(
                                pvp[0:65, :],
                                lhsT=v_buf[kvb][:, hg * 65 : hg * 65 + 65],
                                rhs=expT[p][kvb][:, h * 512 : (h + 1) * 512],
                                start=(kvb == 0),
                                stop=(kvb == 7),
                            )
                        emit_norm(p, h, pvp)

                def emit_oproj():
                    pools = [(psA, "proj"), (psS, "sps"), (psV, "pv")]
                    for r, (ch, lb) in enumerate(
                        [(c, l) for c in range(2) for l in range(4)]
                    ):
                        pool, tag = pools[r % 3]
                        ops = pool.tile([128, 512], F32, tag=tag, name=f"ops{ch}{lb}")
                        for hcb in range(8):
                            nc.tensor.matmul(
                                ops[:],
                                lhsT=multi[hcb][:, lb * 128 : (lb + 1) * 128],
                                rhs=wo_t[ch][:, hcb, :],
                                start=(hcb == 0),
                                stop=False,
                            )
                        nc.tensor.matmul(
                            ops[:],
                            lhsT=ones_r[0:1, :],
                            rhs=bor[0:1, ch * 512 : (ch + 1) * 512],
                            start=False,
                            stop=True,
                        )
                        osb = small.tile([128, 512], F32, tag="outsb", name=f"ob{ch}{lb}")
                        nc.vector.tensor_copy(osb[:], ops[:])
                        nc.sync.dma_start(
                            out[lb * 128 : (lb + 1) * 128, ch * 512 : (ch + 1) * 512],
                            osb[:],
                        )

                emit_small_consts()
                emit_q(0)
                emit_xct()
                emit_k(0)
                emit_late_consts()
                emit_vproj()
                emit_sblock(0, pv_pair=None, next_pair=1)
                for p in range(1, NP):
                    emit_sblock(
                        p,
                        pv_pair=p - 1,
                        next_pair=(p + 1 if p + 1 < NP else None),
                        pv2_pair=(NP - 1 if p == NP - 1 else None),
                    )
                    if p == 5:
                        emit_wo_loads()
                emit_oproj()

            with nc.allow_low_precision(reason="fp32r kernel"):
                for _rep in range(reps):
                    emit_iteration()

    nc.compile()
    return nc


def _marshal(inputs):
    q = np.ascontiguousarray(np.asarray(inputs["queries"], dtype=np.float32))
    c = np.ascontiguousarray(np.asarray(inputs["context"], dtype=np.float32))
    Wq = np.asarray(inputs["Wq"], dtype=np.float32)
    Wk = np.asarray(inputs["Wk"], dtype=np.float32)
    Wv = np.asarray(inputs["Wv"], dtype=np.float32)
    Wo = np.ascontiguousarray(np.asarray(inputs["Wo"], dtype=np.float32))
    bq = np.asarray(inputs["bq"], dtype=np.float32)
    bk = np.asarray(inputs["bk"], dtype=np.float32)
    bv = np.asarray(inputs["bv"], dtype=np.float32)
    bo = np.asarray(inputs["bo"], dtype=np.float32)

    wq_flat = np.ascontiguousarray(Wq.transpose(1, 0, 2).reshape(D, H * HD))
    wk_flat = np.ascontiguousarray(Wk.transpose(1, 0, 2).reshape(D, H * HD))
    wv_aug = np.zeros((D, H, 65), np.float32)
    wv_aug[:, :, :64] = Wv.transpose(1, 0, 2).reshape(D, H, HD)
    wv_aug = np.ascontiguousarray(wv_aug.reshape(D, H * 65))
    bv_aug = np.full((H, 65), 1.0, np.float32)
    bv_aug[:, :64] = bv.reshape(H, HD)
    bv_aug = np.ascontiguousarray(bv_aug.reshape(1, H * 65))

    bq_cols = np.ascontiguousarray(bq.reshape(NP, 128).T)
    bk_cols = np.ascontiguousarray(bk.reshape(NP, 128).T)
    shared = {
        "wq": wq_flat,
        "wk": wk_flat,
        "wv": wv_aug,
        "wo": Wo,
        "bq": bq_cols,
        "bk": bk_cols,
        "bvrow": bv_aug,
        "borow": np.ascontiguousarray(bo.reshape(1, D)),
        "onesd": np.ones((1, 128), np.float32),
    }
    in_maps = []
    for core in range(NCORES):
        b, half = core // 2, core % 2
        m = dict(shared)
        m["xqt"] = np.ascontiguousarray(q[b].T[:, half * LQ_C : (half + 1) * LQ_C])
        m["xct"] = np.ascontiguousarray(c[b].T)
        in_maps.append(m)
    return in_maps


def kernel(**inputs):
    global _PROGRAM
    if _PROGRAM is None:
        _PROGRAM = build_program()
    in_maps = _marshal(inputs)
    res = run_bass_kernel_spmd(_PROGRAM, in_maps, list(range(NCORES)))
    out = np.empty((B, L, D), np.float32)
    residual = np.empty((B, L, H * HD), np.float32)
    for core in range(NCORES):
        b, half = core // 2, core % 2
        sl = slice(half * LQ_C, (half + 1) * LQ_C)
        out[b, sl, :] = res.results[core]["out"]
        residual[b, sl, :] = res.results[core]["resid"].T
    return out, residual



# revision 4
# speedup vs baseline: 1.3146x; 1.3146x over previous
"""Trainium2 Bass kernel for nn_MedPoseAttention (multi-head cross-attention).

Full inputs in, full outputs out. Sharding: 8 cores = 4 batches x 2 query-row
halves. Each core computes one batch's K/V projections over the full context
(replicated within the pair) and attention + output projection for its 512
query rows, all 16 heads. No cross-core communication.

Per-core dataflow:
  Q/K/V/O projections in bf16 (weights + activations host-packed to bf16,
  fully-contiguous SBUF-layout DMAs).
  scores = k8.T @ q8 in fp8(e4m3) DoubleRow mode: contraction (p,2)-packed,
  kT8 zero-padded in the second k-slot, qT8 broadcast (stride-0) - 2x rate.
  exp on ScalarE -> bf16; PV with exp-block stationary, streaming [v|1]
  (F=65): out [q,65] accumulated over kv; denominator rides col 64.
  norm fused into PSUM read (reciprocal + tensor_scalar mult) -> bf16,
  PE-transposed back to [m,q] for the output projection.
  V bias folded into the output bias on host (bo2 = bv @ Wo + bo); O bias
  applied via a partition-broadcast add on the PSUM->SBUF copy.
"""

import sys

if "/opt/trn_rl_repo" not in sys.path:
    sys.path.insert(0, "/opt/trn_rl_repo")

import numpy as np
import ml_dtypes

import concourse.bass as bass  # noqa: F401
import concourse.mybir as mybir
from concourse import bacc, tile
from concourse.bass_utils import run_bass_kernel_spmd
from concourse.masks import make_identity

F32 = mybir.dt.float32
BF16 = mybir.dt.bfloat16
FP8 = mybir.dt.float8e4
MULT = mybir.AluOpType.mult
ADD = mybir.AluOpType.add
EXP = mybir.ActivationFunctionType.Exp
COPY = mybir.ActivationFunctionType.Copy
DR = mybir.MatmulPerfMode.DoubleRow

NPBF = ml_dtypes.bfloat16
NPE4 = ml_dtypes.float8_e4m3

B, L, D, H, HD = 4, 1024, 1024, 16, 64
NCORES = 8
LQ_C = 512  # query rows per core
NP = H // 2  # head pairs
SCALE = 0.125  # 1/sqrt(HD)

_PROGRAM = None


def build_program():
    nc = bacc.Bacc("TRN2", target_bir_lowering=False, debug=False, num_devices=NCORES)

    xq_d = nc.dram_tensor("xq_d", [128, 8 * LQ_C], BF16, kind="ExternalInput").ap()
    xc_d = nc.dram_tensor("xc_d", [128, 8 * L], BF16, kind="ExternalInput").ap()
    wq_d = nc.dram_tensor("wq_d", [128, 8192], BF16, kind="ExternalInput").ap()
    wk_d = nc.dram_tensor("wk_d", [128, 8192], BF16, kind="ExternalInput").ap()
    wv_d = nc.dram_tensor("wv_d", [128, 8192], BF16, kind="ExternalInput").ap()
    wo_d = nc.dram_tensor("wo_d", [128, 8192], BF16, kind="ExternalInput").ap()
    bq_d = nc.dram_tensor("bq_d", [128, NP], F32, kind="ExternalInput").ap()
    bk_d = nc.dram_tensor("bk_d", [128, NP], F32, kind="ExternalInput").ap()
    bo2_d = nc.dram_tensor("bo2_d", [1, D], F32, kind="ExternalInput").ap()

    out_d = nc.dram_tensor("out_d", [LQ_C, D], F32, kind="ExternalOutput").ap()
    res_d = nc.dram_tensor("res_d", [H * HD, LQ_C], BF16, kind="ExternalOutput").ap()

    xq_v = xq_d.rearrange("p (db j) -> p db j", db=8)
    xc_v = xc_d.rearrange("p (db j) -> p db j", db=8)
    wv_v = wv_d.rearrange("p (db c) -> p db c", db=8)
    wo_v = wo_d.rearrange("p (hb c) -> p hb c", hb=8)

    with nc.allow_low_precision(reason="bf16/fp8 kernel"), tile.TileContext(nc) as tc:
        with (
            tc.tile_pool(name="persist", bufs=1) as persist,
            tc.tile_pool(name="wq_p", bufs=2) as wq_pool,
            tc.tile_pool(name="wk_p", bufs=2) as wk_pool,
            tc.tile_pool(name="wv_p", bufs=2) as wv_pool,
            tc.tile_pool(name="qt_p", bufs=2) as qt_pool,
            tc.tile_pool(name="qt8_p", bufs=2) as qt8_pool,
            tc.tile_pool(name="exp_p", bufs=18) as exp_pool,
            tc.tile_pool(name="small", bufs=2) as small,
            tc.tile_pool(name="psP", bufs=1, space="PSUM") as psP,
            tc.tile_pool(name="psS", bufs=2, space="PSUM") as psS,
            tc.tile_pool(name="psV", bufs=2, space="PSUM") as psV,
        ):
            # ---- persistent tiles ----
            xq_all = persist.tile([128, 8, LQ_C], BF16, tag="xq", name="xq_all")
            xc_all = persist.tile([128, 8, L], BF16, tag="xc", name="xc_all")
            v_all = persist.tile([128, 128, 65], BF16, tag="vb", name="v_all")
            k8 = [
                persist.tile([128, 2, L], FP8, tag=f"k8{i}", name=f"k8{i}")
                for i in range(2)
            ]
            ident = persist.tile([128, 128], BF16, tag="id", name="ident")
            bo_b = persist.tile([128, D], F32, tag="bo_b", name="bo_b")
            bo2_sb = persist.tile([1, D], F32, tag="bo2", name="bo2_sb")
            bq_sb = persist.tile([128, NP], F32, tag="bq", name="bq_sb")
            bk_sb = persist.tile([128, NP], F32, tag="bk", name="bk_sb")
            mt = [
                persist.tile([128, LQ_C], BF16, tag=f"mt{p}", name=f"mt{p}")
                for p in range(NP)
            ]
            wo_t = [
                persist.tile([128, 8, 512], BF16, tag=f"wo{c}", name=f"wo{c}")
                for c in range(2)
            ]

            qT = [None] * NP
            qT8 = [None] * NP
            wq_t = [None] * NP
            wk_t = [None] * NP
            ets = [[None] * 8 for _ in range(NP)]

            def emit_preamble():
                # first-needed first: wq0 + xq feed the very first matmuls
                wq_t[0] = wq_pool.tile([128, 8, 128], BF16, tag="wq", name="wq0")
                nc.sync.dma_start(wq_t[0][:], wq_d[:, 0:1024].rearrange("p (db m) -> p db m", db=8))
                nc.scalar.dma_start(xq_all[:, 0:4, :], xq_v[:, 0:4, :])
                nc.scalar.dma_start(xq_all[:, 4:8, :], xq_v[:, 4:8, :])
                wk_t[0] = wk_pool.tile([128, 8, 128], BF16, tag="wk", name="wk0")
                nc.sync.dma_start(wk_t[0][:], wk_d[:, 0:1024].rearrange("p (db m) -> p db m", db=8))
                nc.gpsimd.dma_start(xc_all[:, 0:2, :], xc_v[:, 0:2, :])
                nc.gpsimd.dma_start(xc_all[:, 2:4, :], xc_v[:, 2:4, :])
                nc.scalar.dma_start(xc_all[:, 4:6, :], xc_v[:, 4:6, :])
                nc.scalar.dma_start(xc_all[:, 6:8, :], xc_v[:, 6:8, :])
                nc.sync.dma_start(bq_sb[:], bq_d[:])
                nc.sync.dma_start(bk_sb[:], bk_d[:])
                nc.sync.dma_start(bo2_sb[:], bo2_d[:])
                nc.vector.memset(k8[0][:, 1, :], 0.0)
                nc.vector.memset(k8[1][:, 1, :], 0.0)
                nc.vector.memset(v_all[:, :, 64:65], 1.0)
                make_identity(nc, ident[:])
                nc.gpsimd.partition_broadcast(bo_b[:], bo2_sb[0:1, :])

            def finish_qproj(pr, qps):
                qT[pr] = qt_pool.tile([128, LQ_C], BF16, tag="qt", name=f"qt{pr}")
                nc.vector.tensor_scalar_add(qT[pr][:], qps[:], bq_sb[:, pr : pr + 1])
                qT8[pr] = qt8_pool.tile([128, LQ_C], FP8, tag="qt8", name=f"qt8{pr}")
                nc.scalar.activation(qT8[pr][:], qT[pr][:], COPY)
                nc.sync.dma_start(res_d[pr * 128 : (pr + 1) * 128, :], qT[pr][:])

            def emit_qproj0():
                qps = psP.tile([128, LQ_C], F32, tag="qps", name="qps0")
                for d in range(8):
                    nc.tensor.matmul(
                        qps[:], lhsT=wq_t[0][:, d, :], rhs=xq_all[:, d, :],
                        start=(d == 0), stop=(d == 7),
                    )
                finish_qproj(0, qps)

            def emit_kproj0():
                for ch in range(2):
                    kps = psP.tile([128, 512], F32, tag="kps", name=f"kps0{ch}")
                    for d in range(8):
                        nc.tensor.matmul(
                            kps[:], lhsT=wk_t[0][:, d, :],
                            rhs=xc_all[:, d, ch * 512 : (ch + 1) * 512],
                            start=(d == 0), stop=(d == 7),
                        )
                    nc.vector.tensor_scalar_add(
                        k8[0][:, 0, ch * 512 : (ch + 1) * 512], kps[:],
                        bk_sb[:, 0:1],
                    )

            def emit_vproj():
                wv_t = wv_pool.tile([128, 8, 256], BF16, tag="wv", name="wv0")
                nc.sync.dma_start(wv_t[:], wv_v[:, :, 0:256])
                for ch in range(4):
                    if ch < 3:
                        wv_n = wv_pool.tile([128, 8, 256], BF16, tag="wv", name=f"wv{ch+1}")
                        nc.sync.dma_start(
                            wv_n[:], wv_v[:, :, (ch + 1) * 256 : (ch + 2) * 256]
                        )
                    for kvb in range(8):
                        vps = psS.tile([128, 256], F32, tag="sps", name=f"vps{ch}{kvb}")
                        for d in range(8):
                            nc.tensor.matmul(
                                vps[:],
                                lhsT=xc_all[:, d, kvb * 128 : (kvb + 1) * 128],
                                rhs=wv_t[:, d, :],
                                start=(d == 0), stop=(d == 7),
                            )
                        nc.vector.tensor_copy(
                            v_all[:, kvb * 16 + ch * 4 : kvb * 16 + ch * 4 + 4, 0:64],
                            vps[:].rearrange("p (h m) -> p h m", h=4),
                        )
                    wv_t = wv_n if ch < 3 else None

            # deferred-transpose state: (pair, h, qb, pvn_tile)
            pending = [None]

            def emit_pv_group(hp, h, qb):
                hg0 = 2 * hp + h
                pv = psV.tile([128, 65], F32, tag="pvmt", name=f"pv{hp}{h}{qb}")
                for kvb in range(8):
                    nc.tensor.matmul(
                        pv[:],
                        lhsT=ets[hp][kvb][:, h * 512 + qb * 128 : h * 512 + (qb + 1) * 128],
                        rhs=v_all[:, kvb * 16 + hg0, :],
                        start=(kvb == 0), stop=(kvb == 7),
                    )
                rcp = small.tile([128, 1], F32, tag="rcp", name=f"rc{hp}{h}{qb}")
                nc.vector.reciprocal(rcp[:], pv[:, 64:65])
                pvn = small.tile([128, 64], BF16, tag="pvn", name=f"pn{hp}{h}{qb}")
                nc.vector.tensor_scalar_mul(pvn[:], pv[:, 0:64], rcp[:])
                return (hp, h, qb, pvn)

            def emit_pv_transpose(entry):
                hp, h, qb, pvn = entry
                mtp = psV.tile([128, 128], BF16, tag="pvmt", name=f"mp{hp}{h}{qb}")
                nc.tensor.transpose(
                    mtp[h * 64 : (h + 1) * 64, :], pvn[:], ident[:],
                    tile_position=(0, h * 64),
                )
                nc.vector.tensor_copy(
                    mt[hp][h * 64 : (h + 1) * 64, qb * 128 : (qb + 1) * 128],
                    mtp[h * 64 : (h + 1) * 64, :],
                )

            def emit_pair_loop(p):
                nxt = p + 1 if p + 1 < NP else None
                k8cur = k8[p % 2]
                if nxt is not None:
                    wq_t[nxt] = wq_pool.tile([128, 8, 128], BF16, tag="wq", name=f"wq{nxt}")
                    nc.sync.dma_start(
                        wq_t[nxt][:],
                        wq_d[:, nxt * 1024 : (nxt + 1) * 1024].rearrange("p (db m) -> p db m", db=8),
                    )
                    wk_t[nxt] = wk_pool.tile([128, 8, 128], BF16, tag="wk", name=f"wk{nxt}")
                    nc.sync.dma_start(
                        wk_t[nxt][:],
                        wk_d[:, nxt * 1024 : (nxt + 1) * 1024].rearrange("p (db m) -> p db m", db=8),
                    )
                    qps = psP.tile([128, LQ_C], F32, tag="qps", name=f"qps{nxt}")
                    k8n = k8[nxt % 2]
                    kps = None
                for s in range(8):
                    # scores (p, s): 4 fp8 DoubleRow matmuls
                    sps = psS.tile([128, 1024], F32, tag="sps", name=f"sps{p}{s}")
                    for h in range(2):
                        for qc in range(2):
                            rhs = (
                                qT8[p][h * 64 : (h + 1) * 64, qc * 256 : (qc + 1) * 256]
                                .unsqueeze(1)
                                .broadcast_to((64, 2, 256))
                            )
                            nc.tensor.matmul(
                                sps[:, h * 512 + qc * 256 : h * 512 + (qc + 1) * 256],
                                lhsT=k8cur[h * 64 : (h + 1) * 64, :, s * 128 : (s + 1) * 128],
                                rhs=rhs,
                                start=True, stop=True,
                                perf_mode=DR,
                            )
                    et = exp_pool.tile([128, 1024], BF16, tag="et", name=f"et{p}{s}")
                    nc.scalar.activation(et[:], sps[:], EXP, scale=SCALE)
                    ets[p][s] = et
                    # deferred transpose + PV group for pair p-1
                    if pending[0] is not None:
                        emit_pv_transpose(pending[0])
                        pending[0] = None
                    if p > 0:
                        pending[0] = emit_pv_group(p - 1, s % 2, s // 2)
                    # next-pair projections
                    if nxt is not None:
                        nc.tensor.matmul(
                            qps[:], lhsT=wq_t[nxt][:, s, :], rhs=xq_all[:, s, :],
                            start=(s == 0), stop=(s == 7),
                        )
                        ch, d0 = divmod(2 * s, 8)
                        if d0 == 0:
                            kps = psP.tile([128, 512], F32, tag="kps", name=f"kps{nxt}{ch}")
                        for d in (d0, d0 + 1):
                            nc.tensor.matmul(
                                kps[:], lhsT=wk_t[nxt][:, d, :],
                                rhs=xc_all[:, d, ch * 512 : (ch + 1) * 512],
                                start=(d == 0), stop=(d == 7),
                            )
                        if d0 + 1 == 7:
                            nc.vector.tensor_scalar_add(
                                k8n[:, 0, ch * 512 : (ch + 1) * 512], kps[:],
                                bk_sb[:, nxt : nxt + 1],
                            )
                if nxt is not None:
                    finish_qproj(nxt, qps)

            def emit_tail():
                # last transposes of pair 6, PV groups of pair 7 interleaved
                # with O-projection blocks (qb-major so O(lb=qb) unblocks)
                def emit_oblock(r):
                    qb, ch = divmod(r, 2)
                    ops = psP.tile(
                        [128, 512], F32, tag=("qps" if r % 2 == 0 else "kps"),
                        name=f"ops{qb}{ch}",
                    )
                    for hcb in range(8):
                        nc.tensor.matmul(
                            ops[:],
                            lhsT=mt[hcb][:, qb * 128 : (qb + 1) * 128],
                            rhs=wo_t[ch][:, hcb, :],
                            start=(hcb == 0), stop=(hcb == 7),
                        )
                    osb = small.tile([128, 512], F32, tag="osb", name=f"ob{qb}{ch}", bufs=3)
                    nc.vector.tensor_tensor(
                        osb[:], ops[:], bo_b[:, ch * 512 : (ch + 1) * 512], op=ADD
                    )
                    nc.sync.dma_start(
                        out_d[qb * 128 : (qb + 1) * 128, ch * 512 : (ch + 1) * 512],
                        osb[:],
                    )

                for qb in range(4):
                    for h in range(2):
                        if pending[0] is not None:
                            emit_pv_transpose(pending[0])
                        pending[0] = emit_pv_group(7, h, qb)
                    if qb > 0:
                        emit_oblock(2 * (qb - 1))
                        emit_oblock(2 * (qb - 1) + 1)
                emit_pv_transpose(pending[0])
                pending[0] = None
                emit_oblock(6)
                emit_oblock(7)

            emit_preamble()
            emit_qproj0()
            emit_kproj0()
            emit_vproj()
            for p in range(NP):
                if p == 5:
                    for c in range(2):
                        nc.gpsimd.dma_start(
                            wo_t[c][:], wo_v[:, :, c * 512 : (c + 1) * 512]
                        )
                emit_pair_loop(p)
            emit_tail()

    nc.compile()
    return nc


def _marshal(inputs):
    q = np.asarray(inputs["queries"], dtype=np.float32)
    c = np.asarray(inputs["context"], dtype=np.float32)
    Wq = np.asarray(inputs["Wq"], dtype=np.float32)
    Wk = np.asarray(inputs["Wk"], dtype=np.float32)
    Wv = np.asarray(inputs["Wv"], dtype=np.float32)
    Wo = np.asarray(inputs["Wo"], dtype=np.float32)
    bq = np.asarray(inputs["bq"], dtype=np.float32)
    bk = np.asarray(inputs["bk"], dtype=np.float32)
    bv = np.asarray(inputs["bv"], dtype=np.float32)
    bo = np.asarray(inputs["bo"], dtype=np.float32)

    def pack_w(W):  # [H, D, HD] -> [128, 8192] bf16 (p, pr, db, m)
        Wt = W.transpose(1, 0, 2).reshape(D, H * HD)
        return np.ascontiguousarray(
            Wt.reshape(8, 128, 8, 128).transpose(1, 2, 0, 3).reshape(128, 8192)
        ).astype(NPBF)

    def pack_rows(Wt):  # [D(rows=8*128), C] -> [128, 8*C]
        C = Wt.shape[1]
        return np.ascontiguousarray(
            Wt.reshape(8, 128, C).transpose(1, 0, 2).reshape(128, 8 * C)
        ).astype(NPBF)

    wq_pk = pack_w(Wq)
    wk_pk = pack_w(Wk)
    wv_pk = pack_rows(Wv.transpose(1, 0, 2).reshape(D, H * HD))
    wo_pk = pack_rows(Wo)

    bq_c = np.ascontiguousarray(bq.reshape(NP, 128).T)
    bk_c = np.ascontiguousarray(bk.reshape(NP, 128).T)
    bo2 = (
        bv.reshape(1, H * HD).astype(np.float64) @ Wo.astype(np.float64)
        + bo.astype(np.float64)
    ).astype(np.float32)

    shared = {
        "wq_d": wq_pk, "wk_d": wk_pk, "wv_d": wv_pk, "wo_d": wo_pk,
        "bq_d": bq_c, "bk_d": bk_c, "bo2_d": bo2,
    }
    in_maps = []
    for core in range(NCORES):
        b, half = core // 2, core % 2
        m = dict(shared)
        xq = q[b].T[:, half * LQ_C : (half + 1) * LQ_C]
        m["xq_d"] = np.ascontiguousarray(
            xq.reshape(8, 128, LQ_C).transpose(1, 0, 2).reshape(128, 8 * LQ_C)
        ).astype(NPBF)
        m["xc_d"] = pack_rows(c[b].T)
        in_maps.append(m)
    return in_maps


def kernel(**inputs):
    global _PROGRAM
    if _PROGRAM is None:
        _PROGRAM = build_program()
    in_maps = _marshal(inputs)
    res = run_bass_kernel_spmd(_PROGRAM, in_maps, list(range(NCORES)))
    out = np.empty((B, L, D), np.float32)
    residual = np.empty((B, L, H * HD), np.float32)
    for core in range(NCORES):
        b, half = core // 2, core % 2
        sl = slice(half * LQ_C, (half + 1) * LQ_C)
        out[b, sl, :] = res.results[core]["out_d"]
        residual[b, sl, :] = res.results[core]["res_d"].astype(np.float32).T
    return out, residual


# revision 11
# speedup vs baseline: 1.3744x; 1.0455x over previous
"""Trainium2 Bass kernel for nn_MedPoseAttention (multi-head cross-attention).

Full inputs in, full outputs out. Sharding: 8 cores = 4 batches x 2 query-row
halves. Each core computes one batch's K/V projections over the full context
(replicated within the pair) and attention + output projection for its 512
query rows, all 16 heads. No cross-core communication.

Per-core dataflow:
  Q/K/V/O projections in bf16 (weights + activations host-packed to bf16,
  fully-contiguous SBUF-layout DMAs).
  scores = k8.T @ q8 in fp8(e4m3) DoubleRow mode: contraction (p,2)-packed,
  kT8 zero-padded in the second k-slot, qT8 broadcast (stride-0) - 2x rate.
  exp on ScalarE -> bf16; PV with exp-block stationary, streaming [v|1]
  (F=65): out [q,65] accumulated over kv; denominator rides col 64.
  norm fused into PSUM read (reciprocal + tensor_scalar mult) -> bf16,
  PE-transposed back to [m,q] for the output projection.
  V bias folded into the output bias on host (bo2 = bv @ Wo + bo); O bias
  applied via a partition-broadcast add on the PSUM->SBUF copy.
"""

import sys

if "/opt/trn_rl_repo" not in sys.path:
    sys.path.insert(0, "/opt/trn_rl_repo")

import numpy as np
import ml_dtypes

import concourse.bass as bass  # noqa: F401
import concourse.mybir as mybir
from concourse import bacc, tile
from concourse.bass_utils import run_bass_kernel_spmd
from concourse.masks import make_identity

F32 = mybir.dt.float32
BF16 = mybir.dt.bfloat16
FP8 = mybir.dt.float8e4
MULT = mybir.AluOpType.mult
ADD = mybir.AluOpType.add
EXP = mybir.ActivationFunctionType.Exp
COPY = mybir.ActivationFunctionType.Copy
DR = mybir.MatmulPerfMode.DoubleRow

NPBF = ml_dtypes.bfloat16
NPE4 = ml_dtypes.float8_e4m3

B, L, D, H, HD = 4, 1024, 1024, 16, 64
NCORES = 8
LQ_C = 512  # query rows per core
NP = H // 2  # head pairs
SCALE = 0.125  # 1/sqrt(HD)

_PROGRAM = None


def build_program():
    nc = bacc.Bacc("TRN2", target_bir_lowering=False, debug=False, num_devices=NCORES)

    xq_d = nc.dram_tensor("xq_d", [128, 8 * LQ_C], BF16, kind="ExternalInput").ap()
    xc_d = nc.dram_tensor("xc_d", [128, 8 * L], BF16, kind="ExternalInput").ap()
    wq_d = nc.dram_tensor("wq_d", [128, 8192], BF16, kind="ExternalInput").ap()
    wk_d = nc.dram_tensor("wk_d", [128, 8192], BF16, kind="ExternalInput").ap()
    wv_d = nc.dram_tensor("wv_d", [128, 8192], BF16, kind="ExternalInput").ap()
    wo_d = nc.dram_tensor("wo_d", [128, 8192], BF16, kind="ExternalInput").ap()
    bq_d = nc.dram_tensor("bq_d", [128, NP], F32, kind="ExternalInput").ap()
    bk_d = nc.dram_tensor("bk_d", [128, NP], F32, kind="ExternalInput").ap()
    bo2_d = nc.dram_tensor("bo2_d", [1, D], F32, kind="ExternalInput").ap()

    out_d = nc.dram_tensor("out_d", [LQ_C, D], F32, kind="ExternalOutput").ap()
    res_d = nc.dram_tensor("res_d", [H * HD, LQ_C], BF16, kind="ExternalOutput").ap()

    xq_v = xq_d.rearrange("p (db j) -> p db j", db=8)
    xc_v = xc_d.rearrange("p (db j) -> p db j", db=8)
    wv_v = wv_d.rearrange("p (db c) -> p db c", db=8)
    wo_v = wo_d.rearrange("p (hb c) -> p hb c", hb=8)

    with nc.allow_low_precision(reason="bf16/fp8 kernel"), tile.TileContext(nc) as tc:
        with (
            tc.tile_pool(name="persist", bufs=1) as persist,
            tc.tile_pool(name="wq_p", bufs=2) as wq_pool,
            tc.tile_pool(name="wk_p", bufs=2) as wk_pool,
            tc.tile_pool(name="wv_p", bufs=2) as wv_pool,
            tc.tile_pool(name="qt_p", bufs=2) as qt_pool,
            tc.tile_pool(name="qt8_p", bufs=2) as qt8_pool,
            tc.tile_pool(name="exp_p", bufs=18) as exp_pool,
            tc.tile_pool(name="small", bufs=2) as small,
            tc.tile_pool(name="psP", bufs=1, space="PSUM") as psP,
            tc.tile_pool(name="psS", bufs=2, space="PSUM") as psS,
            tc.tile_pool(name="psV", bufs=2, space="PSUM") as psV,
        ):
            # ---- persistent tiles ----
            xq_all = persist.tile([128, 8, LQ_C], BF16, tag="xq", name="xq_all")
            xc_all = persist.tile([128, 8, L], BF16, tag="xc", name="xc_all")
            v_all = persist.tile([128, 128, 65], BF16, tag="vb", name="v_all")
            k8 = [
                persist.tile([128, 2, L], FP8, tag=f"k8{i}", name=f"k8{i}")
                for i in range(2)
            ]
            ident = persist.tile([128, 128], BF16, tag="id", name="ident")
            bo_b = persist.tile([128, D], F32, tag="bo_b", name="bo_b")
            bo2_sb = persist.tile([1, D], F32, tag="bo2", name="bo2_sb")
            bq_sb = persist.tile([128, NP], F32, tag="bq", name="bq_sb")
            bk_sb = persist.tile([128, NP], F32, tag="bk", name="bk_sb")
            mt = [
                persist.tile([128, LQ_C], BF16, tag=f"mt{p}", name=f"mt{p}")
                for p in range(NP)
            ]
            wo_t = [
                persist.tile([128, 8, 512], BF16, tag=f"wo{c}", name=f"wo{c}")
                for c in range(2)
            ]

            qT = [None] * NP
            qT8 = [None] * NP
            wq_t = [None] * NP
            wk_t = [None] * NP
            ets = [[None] * 8 for _ in range(NP)]

            def emit_preamble():
                # first-needed first, interleaved so Q/K matmuls can ride the
                # chunk arrivals (shared DMA pool serializes transfers)
                wq_t[0] = wq_pool.tile([128, 8, 128], BF16, tag="wq", name="wq0")
                nc.sync.dma_start(wq_t[0][:], wq_d[:, 0:1024].rearrange("p (db m) -> p db m", db=8))
                nc.scalar.dma_start(xq_all[:, 0:2, :], xq_v[:, 0:2, :])
                wk_t[0] = wk_pool.tile([128, 8, 128], BF16, tag="wk", name="wk0")
                nc.sync.dma_start(wk_t[0][:], wk_d[:, 0:1024].rearrange("p (db m) -> p db m", db=8))
                nc.scalar.dma_start(xc_all[:, 0:2, :], xc_v[:, 0:2, :])
                nc.scalar.dma_start(xq_all[:, 2:4, :], xq_v[:, 2:4, :])
                nc.sync.dma_start(xc_all[:, 2:4, :], xc_v[:, 2:4, :])
                nc.scalar.dma_start(xq_all[:, 4:6, :], xq_v[:, 4:6, :])
                nc.sync.dma_start(xc_all[:, 4:6, :], xc_v[:, 4:6, :])
                nc.scalar.dma_start(xq_all[:, 6:8, :], xq_v[:, 6:8, :])
                nc.sync.dma_start(xc_all[:, 6:8, :], xc_v[:, 6:8, :])
                nc.vector.memset(k8[0][:, 1, :], 0.0)
                nc.vector.memset(k8[1][:, 1, :], 0.0)
                nc.vector.memset(v_all[:, :, 64:65], 1.0)
                make_identity(nc, ident[:])

            def emit_late_consts():
                nc.sync.dma_start(bq_sb[:], bq_d[:])
                nc.sync.dma_start(bk_sb[:], bk_d[:])
                nc.sync.dma_start(bo2_sb[:], bo2_d[:])
                nc.gpsimd.partition_broadcast(bo_b[:], bo2_sb[0:1, :])

            def finish_qproj(pr, qps):
                qT[pr] = qt_pool.tile([128, LQ_C], BF16, tag="qt", name=f"qt{pr}")
                nc.vector.tensor_scalar_add(qT[pr][:], qps[:], bq_sb[:, pr : pr + 1])
                qT8[pr] = qt8_pool.tile([128, LQ_C], FP8, tag="qt8", name=f"qt8{pr}")
                nc.scalar.activation(qT8[pr][:], qT[pr][:], COPY)
                nc.sync.dma_start(res_d[pr * 128 : (pr + 1) * 128, :], qT[pr][:])

            def emit_qk0():
                # Q and K(ch0) d-matmuls interleaved to ride xq/xc chunk DMAs
                qps = psP.tile([128, LQ_C], F32, tag="qps", name="qps0")
                kps = psP.tile([128, 512], F32, tag="kps", name="kps00")
                for dp in range(4):
                    for d in (2 * dp, 2 * dp + 1):
                        nc.tensor.matmul(
                            qps[:], lhsT=wq_t[0][:, d, :], rhs=xq_all[:, d, :],
                            start=(d == 0), stop=(d == 7),
                        )
                    for d in (2 * dp, 2 * dp + 1):
                        nc.tensor.matmul(
                            kps[:], lhsT=wk_t[0][:, d, :],
                            rhs=xc_all[:, d, 0:512],
                            start=(d == 0), stop=(d == 7),
                        )
                finish_qproj(0, qps)
                nc.vector.tensor_scalar_add(
                    k8[0][:, 0, 0:512], kps[:], bk_sb[:, 0:1]
                )
                kps = psP.tile([128, 512], F32, tag="kps", name="kps01")
                for d in range(8):
                    nc.tensor.matmul(
                        kps[:], lhsT=wk_t[0][:, d, :],
                        rhs=xc_all[:, d, 512:1024],
                        start=(d == 0), stop=(d == 7),
                    )
                nc.vector.tensor_scalar_add(
                    k8[0][:, 0, 512:1024], kps[:], bk_sb[:, 0:1]
                )

            def emit_vproj():
                wv_t = wv_pool.tile([128, 8, 256], BF16, tag="wv", name="wv0")
                nc.sync.dma_start(wv_t[:], wv_v[:, :, 0:256])
                for ch in range(4):
                    if ch < 3:
                        wv_n = wv_pool.tile([128, 8, 256], BF16, tag="wv", name=f"wv{ch+1}")
                        nc.sync.dma_start(
                            wv_n[:], wv_v[:, :, (ch + 1) * 256 : (ch + 2) * 256]
                        )
                    for kvb in range(8):
                        vps = psS.tile([128, 256], F32, tag="sps", name=f"vps{ch}{kvb}")
                        for d in range(8):
                            nc.tensor.matmul(
                                vps[:],
                                lhsT=xc_all[:, d, kvb * 128 : (kvb + 1) * 128],
                                rhs=wv_t[:, d, :],
                                start=(d == 0), stop=(d == 7),
                            )
                        nc.vector.tensor_copy(
                            v_all[:, kvb * 16 + ch * 4 : kvb * 16 + ch * 4 + 4, 0:64],
                            vps[:].rearrange("p (h m) -> p h m", h=4),
                        )
                    wv_t = wv_n if ch < 3 else None

            # deferred-transpose state: (pair, h, qb, pvn_tile)
            pending = [None]

            def emit_pv_group(hp, h, qb):
                hg0 = 2 * hp + h
                pv = psV.tile([128, 65], F32, tag="pvmt", name=f"pv{hp}{h}{qb}")
                for kvb in range(8):
                    nc.tensor.matmul(
                        pv[:],
                        lhsT=ets[hp][kvb][:, h * 512 + qb * 128 : h * 512 + (qb + 1) * 128],
                        rhs=v_all[:, kvb * 16 + hg0, :],
                        start=(kvb == 0), stop=(kvb == 7),
                    )
                rcp = small.tile([128, 1], F32, tag="rcp", name=f"rc{hp}{h}{qb}")
                nc.vector.reciprocal(rcp[:], pv[:, 64:65])
                pvn = small.tile([128, 64], BF16, tag="pvn", name=f"pn{hp}{h}{qb}")
                nc.vector.tensor_scalar_mul(pvn[:], pv[:, 0:64], rcp[:])
                return (hp, h, qb, pvn)

            def emit_pv_transpose(entry):
                hp, h, qb, pvn = entry
                mtp = psV.tile([128, 128], BF16, tag="pvmt", name=f"mp{hp}{h}{qb}")
                nc.tensor.transpose(
                    mtp[h * 64 : (h + 1) * 64, :], pvn[:], ident[:],
                    tile_position=(0, h * 64),
                )
                nc.vector.tensor_copy(
                    mt[hp][h * 64 : (h + 1) * 64, qb * 128 : (qb + 1) * 128],
                    mtp[h * 64 : (h + 1) * 64, :],
                )

            osb_t = [None] * 8  # O-proj partial sums parked in SBUF

            def emit_obegin(r):
                # first 6 hcb of O block r=(qb,ch), interleaved into pair-7;
                # partial + output bias parked in SBUF
                qb, ch = divmod(r, 2)
                ops = psP.tile(
                    [128, 512], F32, tag=("qps" if r % 2 == 0 else "kps"),
                    name=f"opsb{qb}{ch}",
                )
                for hcb in range(6):
                    nc.tensor.matmul(
                        ops[:],
                        lhsT=mt[hcb][:, qb * 128 : (qb + 1) * 128],
                        rhs=wo_t[ch][:, hcb, :],
                        start=(hcb == 0), stop=(hcb == 5),
                    )
                osb_t[r] = small.tile(
                    [128, 512], F32, tag="osb", name=f"ob{qb}{ch}", bufs=8
                )
                nc.vector.tensor_tensor(
                    osb_t[r][:], ops[:], bo_b[:, ch * 512 : (ch + 1) * 512], op=ADD
                )

            def emit_ofinish(r):
                qb, ch = divmod(r, 2)
                ops = psP.tile(
                    [128, 512], F32, tag=("qps" if r % 2 == 0 else "kps"),
                    name=f"opsf{qb}{ch}",
                )
                for hcb in (6, 7):
                    nc.tensor.matmul(
                        ops[:],
                        lhsT=mt[hcb][:, qb * 128 : (qb + 1) * 128],
                        rhs=wo_t[ch][:, hcb, :],
                        start=(hcb == 6), stop=(hcb == 7),
                    )
                nc.vector.tensor_tensor(osb_t[r][:], osb_t[r][:], ops[:], op=ADD)
                nc.sync.dma_start(
                    out_d[qb * 128 : (qb + 1) * 128, ch * 512 : (ch + 1) * 512],
                    osb_t[r][:],
                )

            def emit_pair_loop(p):
                nxt = p + 1 if p + 1 < NP else None
                k8cur = k8[p % 2]
                if nxt is not None:
                    wq_t[nxt] = wq_pool.tile([128, 8, 128], BF16, tag="wq", name=f"wq{nxt}")
                    nc.sync.dma_start(
                        wq_t[nxt][:],
                        wq_d[:, nxt * 1024 : (nxt + 1) * 1024].rearrange("p (db m) -> p db m", db=8),
                    )
                    wk_t[nxt] = wk_pool.tile([128, 8, 128], BF16, tag="wk", name=f"wk{nxt}")
                    nc.sync.dma_start(
                        wk_t[nxt][:],
                        wk_d[:, nxt * 1024 : (nxt + 1) * 1024].rearrange("p (db m) -> p db m", db=8),
                    )
                    qps = psP.tile([128, LQ_C], F32, tag="qps", name=f"qps{nxt}")
                    k8n = k8[nxt % 2]
                    kps = None
                for s in range(8):
                    # scores (p, s): 4 fp8 DoubleRow matmuls
                    sps = psS.tile([128, 1024], F32, tag="sps", name=f"sps{p}{s}")
                    for h in range(2):
                        for qc in range(2):
                            rhs = (
                                qT8[p][h * 64 : (h + 1) * 64, qc * 256 : (qc + 1) * 256]
                                .unsqueeze(1)
                                .broadcast_to((64, 2, 256))
                            )
                            nc.tensor.matmul(
                                sps[:, h * 512 + qc * 256 : h * 512 + (qc + 1) * 256],
                                lhsT=k8cur[h * 64 : (h + 1) * 64, :, s * 128 : (s + 1) * 128],
                                rhs=rhs,
                                start=True, stop=True,
                                perf_mode=DR,
                            )
                    et = exp_pool.tile([128, 1024], BF16, tag="et", name=f"et{p}{s}")
                    nc.scalar.activation(et[:], sps[:], EXP, scale=SCALE)
                    ets[p][s] = et
                    # deferred transpose + PV group for pair p-1
                    if pending[0] is not None:
                        emit_pv_transpose(pending[0])
                        pending[0] = None
                    if p > 0:
                        pending[0] = emit_pv_group(p - 1, s % 2, s // 2)
                    if p == NP - 1:
                        emit_obegin(s)
                    # next-pair projections
                    if nxt is not None:
                        nc.tensor.matmul(
                            qps[:], lhsT=wq_t[nxt][:, s, :], rhs=xq_all[:, s, :],
                            start=(s == 0), stop=(s == 7),
                        )
                        ch, d0 = divmod(2 * s, 8)
                        if d0 == 0:
                            kps = psP.tile([128, 512], F32, tag="kps", name=f"kps{nxt}{ch}")
                        for d in (d0, d0 + 1):
                            nc.tensor.matmul(
                                kps[:], lhsT=wk_t[nxt][:, d, :],
                                rhs=xc_all[:, d, ch * 512 : (ch + 1) * 512],
                                start=(d == 0), stop=(d == 7),
                            )
                        if d0 + 1 == 7:
                            nc.vector.tensor_scalar_add(
                                k8n[:, 0, ch * 512 : (ch + 1) * 512], kps[:],
                                bk_sb[:, nxt : nxt + 1],
                            )
                if nxt is not None:
                    finish_qproj(nxt, qps)

            def emit_tail():
                # PV groups of pair 7 (qb-major) interleaved with the O-block
                # finishers (hcb 6,7 + SBUF accumulate + store)
                for qb in range(4):
                    for h in range(2):
                        if pending[0] is not None:
                            emit_pv_transpose(pending[0])
                        pending[0] = emit_pv_group(7, h, qb)
                    if qb > 0:
                        emit_ofinish(2 * (qb - 1))
                        emit_ofinish(2 * (qb - 1) + 1)
                emit_pv_transpose(pending[0])
                pending[0] = None
                emit_ofinish(6)
                emit_ofinish(7)

            emit_preamble()
            emit_late_consts()
            emit_qk0()
            emit_vproj()
            for p in range(NP):
                if p == 5:
                    for c in range(2):
                        nc.sync.dma_start(
                            wo_t[c][:], wo_v[:, :, c * 512 : (c + 1) * 512]
                        )
                emit_pair_loop(p)
            emit_tail()

    nc.compile()
    return nc


def _marshal(inputs):
    q = np.asarray(inputs["queries"], dtype=np.float32)
    c = np.asarray(inputs["context"], dtype=np.float32)
    Wq = np.asarray(inputs["Wq"], dtype=np.float32)
    Wk = np.asarray(inputs["Wk"], dtype=np.float32)
    Wv = np.asarray(inputs["Wv"], dtype=np.float32)
    Wo = np.asarray(inputs["Wo"], dtype=np.float32)
    bq = np.asarray(inputs["bq"], dtype=np.float32)
    bk = np.asarray(inputs["bk"], dtype=np.float32)
    bv = np.asarray(inputs["bv"], dtype=np.float32)
    bo = np.asarray(inputs["bo"], dtype=np.float32)

    def pack_w(W):  # [H, D, HD] -> [128, 8192] bf16 (p, pr, db, m)
        Wt = W.transpose(1, 0, 2).reshape(D, H * HD)
        return np.ascontiguousarray(
            Wt.reshape(8, 128, 8, 128).transpose(1, 2, 0, 3).reshape(128, 8192)
        ).astype(NPBF)

    def pack_rows(Wt):  # [D(rows=8*128), C] -> [128, 8*C]
        C = Wt.shape[1]
        return np.ascontiguousarray(
            Wt.reshape(8, 128, C).transpose(1, 0, 2).reshape(128, 8 * C)
        ).astype(NPBF)

    wq_pk = pack_w(Wq)
    wk_pk = pack_w(Wk)
    wv_pk = pack_rows(Wv.transpose(1, 0, 2).reshape(D, H * HD))
    wo_pk = pack_rows(Wo)

    bq_c = np.ascontiguousarray(bq.reshape(NP, 128).T)
    bk_c = np.ascontiguousarray(bk.reshape(NP, 128).T)
    bo2 = (
        bv.reshape(1, H * HD).astype(np.float64) @ Wo.astype(np.float64)
        + bo.astype(np.float64)
    ).astype(np.float32)

    shared = {
        "wq_d": wq_pk, "wk_d": wk_pk, "wv_d": wv_pk, "wo_d": wo_pk,
        "bq_d": bq_c, "bk_d": bk_c, "bo2_d": bo2,
    }
    in_maps = []
    for core in range(NCORES):
        b, half = core // 2, core % 2
        m = dict(shared)
        xq = q[b].T[:, half * LQ_C : (half + 1) * LQ_C]
        m["xq_d"] = np.ascontiguousarray(
            xq.reshape(8, 128, LQ_C).transpose(1, 0, 2).reshape(128, 8 * LQ_C)
        ).astype(NPBF)
        m["xc_d"] = pack_rows(c[b].T)
        in_maps.append(m)
    return in_maps


def kernel(**inputs):
    global _PROGRAM
    if _PROGRAM is None:
        _PROGRAM = build_program()
    in_maps = _marshal(inputs)
    res = run_bass_kernel_spmd(_PROGRAM, in_maps, list(range(NCORES)))
    out = np.empty((B, L, D), np.float32)
    residual = np.empty((B, L, H * HD), np.float32)
    for core in range(NCORES):
        b, half = core // 2, core % 2
        sl = slice(half * LQ_C, (half + 1) * LQ_C)
        out[b, sl, :] = res.results[core]["out_d"]
        residual[b, sl, :] = res.results[core]["res_d"].astype(np.float32).T
    return out, residual
